# revision 37
# baseline (speedup 1.0000x reference)
"""Trainium2 Bass kernel for nn_AveragePoolingClassLoss.

Reference computation (per image):
  pred = softmax(logits[:, :5], axis=1)            # drop background ch 5
  idx  = argmax_c pred                             # per-pixel class
  s_c  = sum of pred[c] over pixels with idx == c  # == sum of per-pixel max prob
  n_c  = count of pixels with idx == c
  agg  = s_c / n_c (0 if n_c == 0)
  loss = BCE(agg, class_gt), mean over (image, class), log clamp -100

Device pipeline (per image, all planes [128, 2048]):
  - logits shipped as bf16 (host cast): halves HBM traffic vs fp32.
  - zmax = max tree over the 5 logit planes; masks g_c = [z_c == zmax]
    for c = 0..3 directly on logits, merged into wide DVE compares
    (class 4 stats by subtraction from totals; bf16 ties double-count,
    negligible — see test).  Compares are DVE-only in walrus codegen.
  - e_c = exp(z_c) via Schraudolph: i16 = round(a*z + b) bitcast to bf16
    ~= exp(z) at ~1% error (averages out over ~52k pixels/class).  Two
    planes on DVE (tensor_scalar 4x mode), three on gpsimd, which keeps
    the ACT engine free for the table work (Ln + final Exp).
  - D = sum_c e_c via PE identity-matmul accumulation into two PSUM
    half-tiles (lets the next image's accumulation start early).
  - m = exp(zmax - ln D) (ACT Ln + ACT Exp, one shared act table set;
    the redundant compiler-inserted table reloads are stripped).  The
    final Exp writes the chunked M1 layout and its accumulator output
    yields per-partition sum_m for free.
  - Segment sums on PE: per 127-pixel chunk, matmul with stationary
    lhsT = [m_chunk | ones-col] and moving rhs = [g0|g1|g2|g3]_chunk;
    the accumulated [128, 508] PSUM block holds S_c partials on block
    diagonals and n_c column totals in row 127 (from the ones column).
    One masked-product + one strided reduce extract S_c and n_c; a
    per-image ones-vector matmul folds cross-partition totals into the
    tail PSUM (partition 127 excluded from S, recovered for n by
    subtracting from the all-partition total).
  - Tiny BCE tail on partition 0 at the end.

Sharding: pure data parallel over the batch: 8 cores x 4 images.  Each
core emits its partial BCE numerator sum; the host sums and scales.
"""

import numpy as np
import ml_dtypes
from contextlib import ExitStack

import concourse.bass as bass
import concourse.bacc as bacc
import concourse.mybir as mybir
import concourse.tile as tile
from concourse import masks
from concourse.bass_utils import run_bass_kernel_spmd

F32 = mybir.dt.float32
BF16 = mybir.dt.bfloat16
I16 = mybir.dt.int16
ALU = mybir.AluOpType
ACTF = mybir.ActivationFunctionType
AXX = mybir.AxisListType.X

N_CORES = 8
IMGS_PER_CORE = 4
N_CLASSES = 5
HW = 512 * 512
P = 128
FD = HW // P             # 2048 free-dim elements per plane
CH = 127                 # pixels per trace chunk (col 127 of lhsT = ones)
NCHUNK = 17              # 16*127 = 2032 + 16 remainder
MAIN = 16 * CH           # 2032
REM = FD - MAIN          # 16
GW = 4 * CH              # 508 rhs cols per chunk
LOG_CLAMP = -100.0

# Schraudolph exp in bf16-bit space: i16 = round(a*z + b), bitcast -> bf16.
# b absorbs the bias shift C=7 (minimizes pipeline error, see test) and
# +0.5 so float->int truncation rounds; a rounding HW conversion only
# shifts C by 0.5, which the error curve tolerates.
SCHR_A = 128.0 / float(np.log(2.0))
SCHR_B = 127.0 * 128.0 - 7.0 + 0.5


def _build_program(repeat: int = 1):
    nc = bacc.Bacc(
        "TRN2",
        target_bir_lowering=False,
        debug=False,
        enable_asserts=False,
        num_devices=N_CORES,
    )

    logits = nc.dram_tensor(
        "logits", [IMGS_PER_CORE, N_CLASSES, 512, 512], BF16, kind="ExternalInput"
    )
    gt = nc.dram_tensor("gt", [IMGS_PER_CORE, N_CLASSES], F32, kind="ExternalInput")
    partial = nc.dram_tensor("partial", [1, 1], F32, kind="ExternalOutput")

    with ExitStack() as ctx:
        tc = ctx.enter_context(tile.TileContext(nc))
        _kernel_body(ctx, tc, logits.ap(), gt.ap(), partial.ap(), repeat)

    nc.compile()
    _dedupe_act_table_loads(nc)
    return nc


def _dedupe_act_table_loads(nc):
    """The act-table placement pass maps each function to the first table
    containing it (Exp -> exp_and_others, Ln -> natural_log), reloading on
    every Exp/Ln alternation.  Both live in natural_log_exp_and_others, so
    one load of that set serves the whole program: retarget the first load
    and drop the rest (they carry no sync info)."""
    from concourse.hw_specs import get_activation_tables

    tables = get_activation_tables(nc.m.arch)
    shared = next(
        i for i, (name, funcs) in enumerate(tables.items())
        if ACTF.Exp in funcs and ACTF.Ln in funcs
    )
    first = True
    for b in nc.m.functions[0].blocks:
        keep = []
        for inst in b.instructions:
            if isinstance(inst, mybir.InstLoadActFuncSet):
                si = inst.sync_info
                assert si is None or (not si.on_wait and not si.on_update)
                if first:
                    inst.act_func_set_id = shared
                    first = False
                    keep.append(inst)
                continue
            keep.append(inst)
        b.instructions[:] = keep


def _kernel_body(ctx, tc, logits, gt, partial, repeat=1):
    nc = tc.nc

    cpool = ctx.enter_context(tc.tile_pool(name="const", bufs=1))
    zpool = ctx.enter_context(tc.tile_pool(name="zplanes", bufs=3))
    epool = ctx.enter_context(tc.tile_pool(name="eplanes", bufs=2))
    wpool = ctx.enter_context(tc.tile_pool(name="work", bufs=2))
    mpool = ctx.enter_context(tc.tile_pool(name="mp", bufs=3))
    dpool = ctx.enter_context(tc.tile_pool(name="dpsum", bufs=1, space="PSUM"))
    apool = ctx.enter_context(tc.tile_pool(name="apsum", bufs=3, space="PSUM"))
    tpool = ctx.enter_context(tc.tile_pool(name="tailps", bufs=1, space="PSUM"))

    # --- constants (built once) ---
    ident = cpool.tile([P, P], BF16, tag="ident")
    masks.make_identity(nc, ident[:])

    # diag mask: 4 horizontal copies of I[:, :127]; row 127 all ones (counts).
    # Partition-127-start APs are illegal, so row 127 is set by adding a
    # broadcast of the identity's last column (1 only at partition 127).
    diagm = cpool.tile([P, GW], BF16, tag="diagm")
    for c in range(4):
        nc.vector.tensor_copy(diagm[:, c * CH:(c + 1) * CH], ident[:, 0:CH])
    mask127f = cpool.tile([P, 1], F32, tag="mask127f")
    nc.vector.tensor_copy(mask127f[:], ident[:, 127:128])
    nc.vector.tensor_scalar_add(diagm[:], diagm[:], mask127f[:])

    onesc = cpool.tile([P, 1], BF16, tag="onesc")
    nc.vector.memset(onesc[:], 1.0)
    # 1 everywhere except partition 127 (excludes the counts row), and a
    # full ones column (for count totals by subtraction)
    ones127 = cpool.tile([P, 1], F32, tag="ones127")
    nc.vector.tensor_scalar(
        out=ones127[:], in0=ident[:, 127:128], scalar1=-1.0, scalar2=1.0,
        op0=ALU.mult, op1=ALU.add,
    )
    onesall = cpool.tile([P, 1], F32, tag="onesall")
    nc.vector.memset(onesall[:], 1.0)

    # M1 buffers: [m chunk | ones-col] x 17.  Chunks 0..15 are fully
    # rewritten every image; only the ones cols and the chunk-16 pad
    # (cols 16..126, m = 0 there) need initialization.
    m1bufs = []
    for s in range(2):
        m1 = cpool.tile([P, NCHUNK * P], BF16, tag=f"m1_{s}")
        nc.gpsimd.memset(m1[:, 16 * P + REM:16 * P + CH], 0.0)
        for k in range(NCHUNK):
            nc.gpsimd.memset(m1[:, k * P + CH:k * P + CH + 1], 1.0)
        m1bufs.append(m1)

    # G4 buffers: [g0|g1|g2|g3] per chunk; only the chunk-16 pad cols
    # (16..126 per class) must be zero — they feed the counts row.
    g4bufs = []
    for s in range(2):
        g4 = cpool.tile([P, NCHUNK * GW], BF16, tag=f"g4_{s}")
        for c in range(4):
            off = 16 * GW + c * CH
            nc.gpsimd.memset(g4[:, off + REM:off + CH], 0.0)
        g4bufs.append(g4)

    # per-image stats: rows 0..126 = S_c partials, row 127 = n_c totals;
    # smst: per-partition sum_m from the ACT accumulator (2 cols/image:
    # main + remainder exp op)
    sstats = cpool.tile([P, IMGS_PER_CORE * 4], F32, tag="sstats")
    smst = cpool.tile([P, 3 * IMGS_PER_CORE], F32, tag="smst")

    tot = tpool.tile([1, 64], F32, tag="tot")

    # prefetch gt and its complement right after the first image's plane
    # loads so the end-of-program BCE chain doesn't wait on a DMA
    gpool = ctx.enter_context(tc.tile_pool(name="gt", bufs=1))
    n20 = IMGS_PER_CORE * N_CLASSES
    gtt = gpool.tile([1, n20], F32, tag="gtt")
    gtc = gpool.tile([1, n20], F32, tag="gtc")

    pools = (zpool, epool, wpool, mpool, dpool, apool)
    consts = (ident, diagm, onesc, ones127, onesall, m1bufs, g4bufs)
    for rep in range(repeat):
        # software pipeline: stage1(i+1) is emitted before stage2(i) so
        # image i+1's D accumulation sits ahead of image i's trace
        # matmuls in the PE queue (those wait on the longer ACT chain)
        state = [None] * IMGS_PER_CORE
        state[0] = _stage1(tc, pools, consts, logits, 0)
        nc.sync.dma_start(out=gtt[:], in_=gt.rearrange("(o i) c -> o (i c)", o=1))
        nc.vector.tensor_scalar(
            out=gtc[:], in0=gtt[:], scalar1=-1.0, scalar2=1.0,
            op0=ALU.mult, op1=ALU.add)
        for i in range(IMGS_PER_CORE):
            if i + 1 < IMGS_PER_CORE:
                state[i + 1] = _stage1(tc, pools, consts, logits, i + 1)
            _stage2(tc, pools, consts, sstats, smst, tot, state[i], i)

    _bce_tail(ctx, tc, tot, sstats, smst, gtt, gtc, partial)


def _stage1(tc, pools, consts, logits, i):
    nc = tc.nc
    zpool, epool, wpool, mpool, dpool, apool = pools
    ident, diagm, onesc, ones127, onesall, m1bufs, g4bufs = consts

    # per-plane DMA into one contiguous tile: [5, 512, 512] -> [128, 5*2048]
    Z = zpool.tile([P, N_CLASSES * FD], BF16, tag="Z")
    for c in range(N_CLASSES):
        src = logits[i, c].rearrange("(p a) b -> p (a b)", p=P)
        nc.sync.dma_start(out=Z[:, c * FD:(c + 1) * FD], in_=src)

    def zc(c):
        return Z[:, c * FD:(c + 1) * FD]

    # exp planes spread across engines so no single engine is the wall
    # under either cost-model or real-hardware throughput assumptions:
    # two Schraudolph planes on DVE (tensor_scalar 4x), two on gpsimd,
    # and one true exp on ACT (which otherwise only does Ln + final Exp).
    E = []
    for c in range(1):
        e = epool.tile([P, FD], I16, tag=f"e{c}")
        nc.vector.tensor_scalar(
            out=e[:], in0=zc(c), scalar1=SCHR_A, scalar2=SCHR_B,
            op0=ALU.mult, op1=ALU.add,
        )
        E.append(e[:].bitcast(BF16))
    for c in range(1, 4):
        e = epool.tile([P, FD], I16, tag=f"e{c}")
        nc.gpsimd.tensor_scalar(
            out=e[:], in0=zc(c), scalar1=SCHR_A, scalar2=SCHR_B,
            op0=ALU.mult, op1=ALU.add,
        )
        E.append(e[:].bitcast(BF16))
    e4 = epool.tile([P, FD], BF16, tag="e4")
    nc.scalar.activation(e4[:], zc(4), ACTF.Exp)
    E.append(e4[:])
    dplanes = E

    # max tree over logit planes (compares are DVE-only on walrus); first
    # level as one double-wide op: [max(z0,z2) | max(z1,z3)]
    t2w = wpool.tile([P, 2 * FD], BF16, tag="t2w")
    nc.vector.tensor_tensor(t2w[:], Z[:, 0:2 * FD], Z[:, 2 * FD:4 * FD], ALU.max)
    t03 = wpool.tile([P, FD], BF16, tag="t03")
    nc.vector.tensor_tensor(t03[:], t2w[:, 0:FD], t2w[:, FD:2 * FD], ALU.max)
    zmax = wpool.tile([P, FD], BF16, tag="zmax")
    nc.vector.tensor_tensor(zmax[:], t03[:], zc(4), ALU.max)

    # masks g_c = [z_c == zmax] for all 4 classes in one wide op each for
    # main chunks and the remainder, broadcasting zmax over the class dim
    g4 = g4bufs[i % 2]
    z4v = Z[:, 0:4 * FD].rearrange("p (c j) -> p c j", c=4)
    nc.vector.tensor_tensor(
        g4[:, 0:16 * GW].rearrange("p (k c4 u) -> p c4 k u", c4=4, u=CH),
        z4v[:, :, 0:MAIN].rearrange("p c (k u) -> p c k u", u=CH),
        zmax[:, 0:MAIN].rearrange("p (c k u) -> p c k u", c=1, u=CH)
            .to_broadcast([P, 4, 16, CH]),
        ALU.is_equal,
    )
    nc.vector.tensor_tensor(
        g4[:, 16 * GW:16 * GW + 4 * CH].rearrange(
            "p (c u) -> p c u", c=4)[:, :, 0:REM],
        z4v[:, :, MAIN:FD],
        zmax[:, MAIN:FD].rearrange("p (c u) -> p c u", c=1)
            .to_broadcast([P, 4, REM]),
        ALU.is_equal,
    )

    # D = sum of the 4 pre-combined planes on PE (identity lhsT, PSUM
    # accumulation), in two half-plane tiles so image i+1's accumulation
    # can start as soon as image i's ln consumed that half
    dh0 = dpool.tile([P, 512], F32, tag="Dh0")
    dh1 = dpool.tile([P, 512], F32, tag="Dh1")
    dh2 = dpool.tile([P, 512], F32, tag="Dh2")
    dh3 = dpool.tile([P, 512], F32, tag="Dh3")
    dh = [dh0, dh1, dh2, dh3]
    nd = len(dplanes)
    for r in range(4):
        for c in range(nd):
            nc.tensor.matmul(
                out=dh[r][:],
                lhsT=ident[:],
                rhs=dplanes[c][:, r * 512:(r + 1) * 512],
                start=(c == 0), stop=(c == nd - 1),
            )

    return zmax, g4, dh


def _stage2(tc, pools, consts, sstats, smst, tot, state, i):
    nc = tc.nc
    zpool, epool, wpool, mpool, dpool, apool = pools
    ident, diagm, onesc, ones127, onesall, m1bufs, g4bufs = consts
    zmax, g4, dh = state

    # m = exp(zmax - ln D), half-granular so the trace matmuls (chunks
    # 0..7 live entirely in half 0) start before half 1 finishes
    tsub = wpool.tile([P, FD], BF16, tag="tsub")
    for h in range(4):
        lnd = wpool.tile([P, 512], BF16, tag=f"lnd{h}")
        nc.scalar.activation(lnd[:], dh[h][:], ACTF.Ln)
        nc.vector.tensor_tensor(
            tsub[:, h * 512:(h + 1) * 512], zmax[:, h * 512:(h + 1) * 512],
            lnd[:], ALU.subtract,
        )

    m1 = m1bufs[i % 2]
    m_v = m1[:].rearrange("p (k u) -> p k u", u=P)[:, :, 0:CH]
    HB = 8 * CH  # 1016: chunks 0..7 need only tsub half 0
    nc.scalar.activation(
        m_v[:, 0:8, :], tsub[:, 0:HB].rearrange("p (k u) -> p k u", u=CH),
        ACTF.Exp, accum_out=smst[:, 3 * i:3 * i + 1],
    )
    nc.scalar.activation(
        m_v[:, 8:16, :], tsub[:, HB:MAIN].rearrange("p (k u) -> p k u", u=CH),
        ACTF.Exp, accum_out=smst[:, 3 * i + 1:3 * i + 2],
    )
    off = 16 * P
    nc.scalar.activation(m1[:, off:off + REM], tsub[:, MAIN:FD], ACTF.Exp,
                         accum_out=smst[:, 3 * i + 2:3 * i + 3])

    # segment sums: S_c diag blocks + n_c row 127
    aps = apool.tile([P, GW], F32, tag="A")
    for k in range(NCHUNK):
        nc.tensor.matmul(
            out=aps[:, 0:GW], lhsT=m1[:, k * P:(k + 1) * P],
            rhs=g4[:, k * GW:(k + 1) * GW],
            start=(k == 0), stop=(k == NCHUNK - 1),
        )

    # extract: masked product then strided reduce -> [128, 4] per image
    mp = mpool.tile([P, GW], F32, tag="MP")
    nc.vector.scalar_tensor_tensor(
        out=mp[:], in0=aps[:, 0:GW], scalar=1.0, in1=diagm[:],
        op0=ALU.mult, op1=ALU.mult,
    )
    nc.vector.reduce_sum(
        sstats[:, i * 4:(i + 1) * 4],
        mp[:].rearrange("p (c u) -> p c u", u=CH),
        axis=AXX,
    )

    # fold this image's cross-partition totals into the tail PSUM now so
    # the end-of-program tail only runs the small BCE vector chain
    nc.tensor.matmul(out=tot[0:1, i * 4:(i + 1) * 4], lhsT=ones127[:],
                     rhs=sstats[:, i * 4:(i + 1) * 4], start=True, stop=True,
                     skip_group_check=True)
    nc.tensor.matmul(out=tot[0:1, 32 + i * 4:32 + (i + 1) * 4], lhsT=onesall[:],
                     rhs=sstats[:, i * 4:(i + 1) * 4], start=True, stop=True,
                     skip_group_check=True)
    nc.tensor.matmul(out=tot[0:1, 16 + i * 3:16 + (i + 1) * 3], lhsT=onesall[:],
                     rhs=smst[:, 3 * i:3 * (i + 1)], start=True, stop=True,
                     skip_group_check=True)


def _bce_tail(ctx, tc, tot, sstats, smst, gtt, gtc, partial):
    """BCE on partition 0 from the pre-accumulated totals in `tot`:
    cols 0:16 = S_c (counts row excluded), 16:28 = sum_m parts,
    32:48 = all-partition totals (S_c + n_c)."""
    nc = tc.nc
    pool = ctx.enter_context(tc.tile_pool(name="tail", bufs=1))
    NI, NC5 = IMGS_PER_CORE, N_CLASSES
    n20 = NI * NC5
    n16 = NI * 4

    stile = pool.tile([1, n16], F32, tag="stile")
    nc.vector.tensor_copy(stile[:], tot[0:1, 0:n16])
    s3 = stile.rearrange("o (i c) -> o i c", c=4)                 # [1,4,4]
    sm = pool.tile([1, NI], F32, tag="sm")                        # [1,4]
    sm2 = tot[0:1, 16:16 + 3 * NI].rearrange("o (i u) -> o i u", u=3)
    nc.vector.reduce_sum(sm[:], sm2, axis=AXX)
    sm = sm[:]
    ntile = pool.tile([1, n16], F32, tag="ntile")
    nc.vector.tensor_tensor(ntile[:], tot[0:1, 32:32 + n16], stile[:],
                            ALU.subtract)
    n3 = ntile.rearrange("o (i c) -> o i c", c=4)                 # [1,4,4]

    ssum = pool.tile([1, NI], F32, tag="ssum")
    nsum = pool.tile([1, NI], F32, tag="nsum")
    nc.vector.reduce_sum(ssum[:], s3, axis=AXX)
    nc.vector.reduce_sum(nsum[:], n3, axis=AXX)

    A = pool.tile([1, n20], F32, tag="A")
    C = pool.tile([1, n20], F32, tag="C")
    A3 = A.rearrange("p (i c) -> p i c", c=NC5)
    C3 = C.rearrange("p (i c) -> p i c", c=NC5)
    nc.vector.tensor_copy(A3[:, :, 0:4], s3)
    nc.vector.tensor_copy(C3[:, :, 0:4], n3)
    nc.vector.tensor_tensor(A3[:, :, 4], sm, ssum[:], ALU.subtract)
    nc.vector.tensor_scalar(
        out=C3[:, :, 4], in0=nsum[:], scalar1=-1.0, scalar2=float(HW),
        op0=ALU.mult, op1=ALU.add,
    )

    nc.vector.tensor_scalar_max(C[:], C[:], 1.0)
    rc = pool.tile([1, n20], F32, tag="rc")
    nc.vector.reciprocal(rc[:], C[:])
    agg = pool.tile([1, n20], F32, tag="agg")
    nc.vector.tensor_tensor(agg[:], A[:], rc[:], ALU.mult)

    logp = pool.tile([1, n20], F32, tag="logp")
    q = pool.tile([1, n20], F32, tag="q")
    logq = pool.tile([1, n20], F32, tag="logq")
    nc.scalar.activation(logp[:], agg[:], ACTF.Ln)
    nc.vector.tensor_scalar(
        out=q[:], in0=agg[:], scalar1=-1.0, scalar2=1.0, op0=ALU.mult, op1=ALU.add
    )
    nc.scalar.activation(logq[:], q[:], ACTF.Ln)

    # fused clamp-then-weight: t = max(log, -100) * gt_term
    t1 = pool.tile([1, n20], F32, tag="t1")
    nc.vector.scalar_tensor_tensor(
        out=t1[:], in0=logp[:], scalar=LOG_CLAMP, in1=gtt[:],
        op0=ALU.max, op1=ALU.mult,
    )
    t2 = pool.tile([1, n20], F32, tag="t2")
    nc.vector.scalar_tensor_tensor(
        out=t2[:], in0=logq[:], scalar=LOG_CLAMP, in1=gtc[:],
        op0=ALU.max, op1=ALU.mult,
    )
    tsum = pool.tile([1, n20], F32, tag="tsum")
    nc.vector.tensor_tensor(tsum[:], t1[:], t2[:], ALU.add)
    out = pool.tile([1, 1], F32, tag="out")
    nc.vector.reduce_sum(out[:], tsum[:], axis=AXX)
    nc.sync.dma_start(out=partial[:], in_=out[:])


_NC_CACHE = {}


def _get_program(repeat: int = 1):
    if repeat not in _NC_CACHE:
        _NC_CACHE[repeat] = _build_program(repeat)
    return _NC_CACHE[repeat]


def _in_maps(segmentation_logits, class_gt):
    seg16 = segmentation_logits[:, :N_CLASSES].astype(ml_dtypes.bfloat16)
    maps = []
    for core in range(N_CORES):
        lo = core * IMGS_PER_CORE
        hi = lo + IMGS_PER_CORE
        maps.append({
            "logits": np.ascontiguousarray(seg16[lo:hi]),
            "gt": np.ascontiguousarray(class_gt[lo:hi], dtype=np.float32),
        })
    return maps


def kernel(segmentation_logits: np.ndarray, class_gt: np.ndarray) -> np.ndarray:
    segmentation_logits = np.ascontiguousarray(segmentation_logits, dtype=np.float32)
    class_gt = np.ascontiguousarray(class_gt, dtype=np.float32)
    B = segmentation_logits.shape[0]
    assert B == N_CORES * IMGS_PER_CORE

    nc = _get_program()
    results = run_bass_kernel_spmd(
        nc, _in_maps(segmentation_logits, class_gt), list(range(N_CORES))
    ).results
    total = sum(float(results[c]["partial"][0, 0]) for c in range(N_CORES))
    loss = -total / (B * N_CLASSES)
    return np.float32(loss)
